# revision 35
# baseline (speedup 1.0000x reference)
"""JacobiKANLinear TRN2 Bass kernel.

out = silu(x) @ W_base^T + einsum('bik,oik->bo', P(tanh(x)), C) + bias

Host-side algebra: Jacobi polynomials (A=B=1, degree 5) are re-expressed in
the monomial basis.  D[o,i,j] = sum_k C[o,i,k] * T[k,j] where T holds the
monomial coefficients of P_k.  The j=0 term is constant (t^0 == 1) and folds
into the bias.  The device then computes 6 feature blocks
[silu(x), t, t^2, t^3, t^4, t^5] (t = tanh(x)) and one fused matmul with
contraction 6*1024 = 6144.  Bias is added during the PSUM->SBUF drain via a
host-broadcast [128, O_SHARD] tile (no PSUM-init matmul).

Sharding (8 cores): 4 batch groups x 2 out-feature halves.  Per core:
batch shard 2048 rows, out shard 512 cols.  Matmuls run in float16
(1 cycle/row on the PE at N=512; fp16 weight loads get the fast-weight-load
path, and fp16 halves the weight DMA and the DVE power-chain cost).
Measured end-to-end quantization error on the actual fixed inputs is
rel 1.4e-3 (vs the 2e-2 gate); accumulation stays fp32 in PSUM.

Scheduling: the v1 kernel spent its first 45us with the PE idle — the
12.6MB weight DMA plus bias/ones DMAs (emitted last) gated every matmul.
This version overlaps the weight-DMA phase with compute: the first 4 batch
chunks' x DMAs are issued up front on the GpSimd DGE queue (x is laid out
host-side so each chunk is a contiguous 4KB-per-partition transfer), the
fp16 weights stream on the Sync queue in pairs, and the 16 chunks are
emitted chunk-major — the Tile scheduler staggers matmuls across chunks as
weight tiles and feature blocks become ready.  A short burst of dummy
matmuls right after the preamble keeps the PE busy so the HAM clock gate
un-throttles (1.2 -> 2.4 GHz) before the real matmul stream begins.
"""
import numpy as np

import concourse.bass as bass
import concourse.mybir as mybir
import concourse.tile as tile
from concourse import bacc
from concourse.bass_utils import run_bass_kernel_spmd

BATCH = 8192
IN_F = 1024
OUT_F = 1024
DEGREE = 5
A = 1.0
B = 1.0

N_CORES = 8
BATCH_GROUPS = 4
OUT_HALVES = 2
B_SHARD = BATCH // BATCH_GROUPS        # 2048
O_SHARD = OUT_F // OUT_HALVES          # 512
N_BLOCKS = DEGREE + 1                  # 6 feature blocks
N_KT = N_BLOCKS * IN_F // 128          # 48 contraction tiles of 128
N_CHUNKS = B_SHARD // 128              # 16 batch chunks per core
IT_PER_BLOCK = IN_F // 128             # 8 in-feature tiles per block
N_WARM = 4                             # chunks interleaved with the w DMA

F32 = mybir.dt.float32
F32R = mybir.dt.float32r
F16 = mybir.dt.float16
W_DMA_GROUP = 4                        # w contraction tiles per DMA


def _jacobi_monomial_matrix():
    """T[k, j] = coefficient of t^j in P_k (A=B=1), float64."""
    T = np.zeros((DEGREE + 1, DEGREE + 1), dtype=np.float64)
    polys = [np.zeros(DEGREE + 1) for _ in range(DEGREE + 1)]
    polys[0][0] = 1.0
    if DEGREE >= 1:
        # 0.5 * (2(A+1) t + (A-B))
        polys[1][1] = A + 1.0
        polys[1][0] = 0.5 * (A - B)
    for k in range(2, DEGREE + 1):
        alpha_n = 2.0 * k * (k + A + B) * (2 * k + A + B - 2)
        beta_n = (2 * k + A + B - 1) * (A ** 2 - B ** 2)
        gamma_n = (2 * k + A + B - 2) * (2 * k + A + B - 1) * (2 * k + A + B)
        delta_n = 2.0 * (k + A - 1) * (k + B - 1) * (2 * k + A + B)
        # P_k = ((beta + alpha t)/gamma) P_{k-1} - (delta/gamma) P_{k-2}
        p = np.zeros(DEGREE + 1)
        p += (beta_n / gamma_n) * polys[k - 1]
        p[1:] += (alpha_n / gamma_n) * polys[k - 1][:-1]
        p -= (delta_n / gamma_n) * polys[k - 2]
        polys[k] = p
    for k in range(DEGREE + 1):
        T[k] = polys[k]
    return T


def _build_nc():
    nc = bacc.Bacc()
    xt_in = nc.declare_dram_parameter(
        "xt", [N_CHUNKS, 128, IT_PER_BLOCK, 128], F16, isOutput=False)
    w_in = nc.declare_dram_parameter(
        "w", [128, N_KT, O_SHARD], F16, isOutput=False)
    biasb_in = nc.declare_dram_parameter(
        "biasb", [128, O_SHARD], F32, isOutput=False)
    out = nc.declare_dram_parameter("out", [B_SHARD, O_SHARD], F16, isOutput=True)

    with tile.TileContext(nc) as tc:
        with tc.tile_pool(name="wpool", bufs=1) as wpool, \
             tc.tile_pool(name="xpool", bufs=4) as xpool, \
             tc.tile_pool(name="fpool", bufs=N_WARM) as fpool, \
             tc.tile_pool(name="opool", bufs=2) as opool, \
             tc.tile_pool(name="psum", bufs=8, space="PSUM") as psum_pool:

            def x_dma(m, eng=None):
                # x/out/bias DMAs go on the (otherwise idle) GpSimd queue so
                # they never serialize behind the weight stream on Sync.
                # Exception: x0 goes FIRST on sync, where it gets the HBM to
                # itself for ~1.3us and lands before the weight stream ramps —
                # chunk 0's silu gates the start of the real matmul stream.
                x_m = xpool.tile([128, IT_PER_BLOCK, 128], F16, tag="x")
                (eng or nc.gpsimd).dma_start(out=x_m[:], in_=xt_in[m])
                return x_m

            def blocks_for(x_m):
                silu_m = fpool.tile([128, IT_PER_BLOCK, 128], F16, tag="silu")
                t_m = fpool.tile([128, IT_PER_BLOCK, 128], F16, tag="t1")
                nc.scalar.activation(
                    silu_m[:], x_m[:], mybir.ActivationFunctionType.Silu)
                nc.scalar.activation(
                    t_m[:], x_m[:], mybir.ActivationFunctionType.Tanh)
                t2_m = fpool.tile([128, IT_PER_BLOCK, 128], F16, tag="t2")
                nc.vector.tensor_mul(t2_m[:], t_m[:], t_m[:])
                t3_m = fpool.tile([128, IT_PER_BLOCK, 128], F16, tag="t3")
                nc.vector.tensor_mul(t3_m[:], t2_m[:], t_m[:])
                t4_m = fpool.tile([128, IT_PER_BLOCK, 128], F16, tag="t4")
                nc.vector.tensor_mul(t4_m[:], t3_m[:], t_m[:])
                t5_m = fpool.tile([128, IT_PER_BLOCK, 128], F16, tag="t5")
                nc.vector.tensor_mul(t5_m[:], t4_m[:], t_m[:])
                return [silu_m, t_m, t2_m, t3_m, t4_m, t5_m]

            def drain(m, ps):
                o_m = opool.tile([128, O_SHARD], F16, tag="o")
                if m == N_CHUNKS - 1:
                    # Last chunk is the kernel tail: slice the bias-add and
                    # ship halves on the (idle, HWDGE) sync queue so the out
                    # DMA overlaps the second half's add.
                    for h in range(2):
                        sl = bass.ts(h, O_SHARD // 2)
                        nc.vector.tensor_add(o_m[:, sl], ps[:, sl], biasb_sb[:, sl])
                        nc.sync.dma_start(
                            out=out[bass.ts(m, 128), sl], in_=o_m[:, sl])
                else:
                    nc.vector.tensor_add(o_m[:], ps[:], biasb_sb[:])
                    nc.gpsimd.dma_start(out=out[bass.ts(m, 128), :], in_=o_m[:])

            # --- PE warm-up operands: memset first so the dummy matmuls can
            # start the moment the PE's preamble ends.
            dum_l = wpool.tile([128, 128], F16)
            dum_r = wpool.tile([128, O_SHARD], F16)
            nc.vector.memset(dum_l[:], 0.0)
            nc.vector.memset(dum_r[:], 0.0)

            # --- DMA issue order.  Early HBM bandwidth is round-robined
            # across everything in flight, so the startup DMAs are all queued
            # on sync in strict NEED order: x0 (gates silu0 -> first matmul),
            # w0/w1, then w pairs with x1..x3 interleaved where the ACT
            # pipeline will want them (silu_c starts ~2.3us after silu_{c-1}).
            # biasb — not needed until the first drain (~25us) — goes last.
            # Single sync ring in strict need-order: chunk 0 alone consumes a
            # 4-tile w group every ~1.7us, which is the full ring rate — so
            # nothing else may transfer until the first three w groups are
            # in.  x1..x3 only gate chunks 1-3 joining (the ACT stagger no
            # longer binds once chunk 0 is streaming).
            w_sb = wpool.tile([128, N_KT, O_SHARD], F16)
            x_tiles = {}
            x_tiles[0] = x_dma(0, eng=nc.sync)
            nc.sync.dma_start(out=w_sb[:, 0:4, :], in_=w_in[:, 0:4, :])
            nc.sync.dma_start(out=w_sb[:, 4:8, :], in_=w_in[:, 4:8, :])
            nc.sync.dma_start(out=w_sb[:, 8:12, :], in_=w_in[:, 8:12, :])
            x_tiles[1] = x_dma(1, eng=nc.sync)
            nc.sync.dma_start(out=w_sb[:, 12:16, :], in_=w_in[:, 12:16, :])
            x_tiles[2] = x_dma(2, eng=nc.sync)
            nc.sync.dma_start(out=w_sb[:, 16:20, :], in_=w_in[:, 16:20, :])
            x_tiles[3] = x_dma(3, eng=nc.sync)
            for kt0 in range(20, N_KT, W_DMA_GROUP):
                nc.sync.dma_start(
                    out=w_sb[:, kt0:kt0 + W_DMA_GROUP, :],
                    in_=w_in[:, kt0:kt0 + W_DMA_GROUP, :])
            biasb_sb = wpool.tile([128, O_SHARD], F32)
            nc.sync.dma_start(out=biasb_sb[:], in_=biasb_in[:])

            # --- PE warm-up: dummy matmuls (K=128, N=512 — full-array
            # activity, which is what the HAM monitors; K=1 dummies don't
            # register) bridge from the end of the preamble to the first real
            # matmul so the HAM clock gate un-throttles (1.2 -> 2.4 GHz)
            # before the real stream begins.
            ps_warm = psum_pool.tile([128, O_SHARD], F32, tag="ps")
            for _ in range(12):
                nc.tensor.matmul(
                    ps_warm[:], dum_l[:], dum_r[:], start=True, stop=True)

            # --- Phase A: warm chunks, staggered-diagonal order.  Chunk c's
            # activations finish ~2.3us after chunk c-1's (ACT is serial), and
            # weight tiles land at ~0.4us each, so emitting matmuls sorted by
            # kt + 5*c keeps the PE stream's static order feasible: chunk 0
            # runs ~5 tiles ahead of chunk 1, etc.
            warm_blocks = []
            warm_ps = []
            for c in range(N_WARM):
                warm_blocks.append(blocks_for(x_tiles[c]))
                ps_c = psum_pool.tile([128, O_SHARD], F32, tag="ps")
                warm_ps.append(ps_c)
            order = sorted(
                ((kt + 5 * c, c, kt) for c in range(N_WARM) for kt in range(N_KT)))
            for _, c, kt in order:
                b, it = kt // IT_PER_BLOCK, kt % IT_PER_BLOCK
                nc.tensor.matmul(
                    warm_ps[c][:], warm_blocks[c][b][:, it, :], w_sb[:, kt, :],
                    start=(kt == 0), stop=(kt == N_KT - 1))
            for c in range(N_WARM):
                drain(c, warm_ps[c])

            # --- Phase B: remaining chunks, chunk-major (weights resident).
            for m in range(N_WARM, N_CHUNKS):
                x_m = x_dma(m)
                blocks = blocks_for(x_m)
                ps = psum_pool.tile([128, O_SHARD], F32, tag="ps")
                for kt in range(N_KT):
                    b, it = kt // IT_PER_BLOCK, kt % IT_PER_BLOCK
                    nc.tensor.matmul(
                        ps[:], blocks[b][:, it, :], w_sb[:, kt, :],
                        start=(kt == 0), stop=(kt == N_KT - 1))
                drain(m, ps)
    nc.finalize()
    return nc


_NC_CACHE = None


def _get_nc():
    global _NC_CACHE
    if _NC_CACHE is None:
        _NC_CACHE = _build_nc()
    return _NC_CACHE


def _prepare_host(x, base_weight, jacobi_coeffs, bias):
    T = _jacobi_monomial_matrix()
    D = np.einsum("oik,kj->oij", jacobi_coeffs.astype(np.float64), T)
    bias_eff = bias.astype(np.float64) + D[:, :, 0].sum(axis=1)

    # W'[f, o]: 6 blocks of IN_F feature rows: silu -> base_weight, t^j -> D_j
    w_full = np.empty((N_BLOCKS * IN_F, OUT_F), dtype=np.float32)
    w_full[0:IN_F] = base_weight.T
    for j in range(1, N_BLOCKS):
        w_full[j * IN_F:(j + 1) * IN_F] = D[:, :, j].T.astype(np.float32)

    w_halves = []
    biasb_halves = []
    for h in range(OUT_HALVES):
        wh = w_full[:, h * O_SHARD:(h + 1) * O_SHARD]
        # SBUF layout [128, N_KT, O_SHARD]: [p, kt, n] = wh[kt*128 + p, n]
        wh = np.ascontiguousarray(
            wh.reshape(N_KT, 128, O_SHARD).transpose(1, 0, 2).astype(np.float16))
        w_halves.append(wh)
        bh = bias_eff[h * O_SHARD:(h + 1) * O_SHARD].astype(np.float32)
        biasb_halves.append(np.ascontiguousarray(
            np.broadcast_to(bh[None, :], (128, O_SHARD))))

    xt_groups = []
    for g in range(BATCH_GROUPS):
        xs = x[g * B_SHARD:(g + 1) * B_SHARD]              # (B_SHARD, IN_F)
        # [m, p, it, b] = xs[m*128 + b, it*128 + p]: per (m, p) the 4KB
        # [it, b] plane is contiguous, so each chunk DMA is 128 fat
        # descriptors instead of 1024 strided 512B ones.
        xt = np.ascontiguousarray(
            xs.reshape(N_CHUNKS, 128, IT_PER_BLOCK, 128)
            .transpose(0, 3, 2, 1).astype(np.float16))
        xt_groups.append(xt)
    return xt_groups, w_halves, biasb_halves


def _in_maps(x, base_weight, jacobi_coeffs, bias):
    xt_groups, w_halves, biasb_halves = _prepare_host(
        x, base_weight, jacobi_coeffs, bias)
    maps = []
    for c in range(N_CORES):
        g, h = c // OUT_HALVES, c % OUT_HALVES
        maps.append({
            "xt": xt_groups[g],
            "w": w_halves[h],
            "biasb": biasb_halves[h],
        })
    return maps


def kernel(x, base_weight, jacobi_coeffs, bias):
    x = np.asarray(x, dtype=np.float32)
    base_weight = np.asarray(base_weight, dtype=np.float32)
    jacobi_coeffs = np.asarray(jacobi_coeffs, dtype=np.float32)
    bias = np.asarray(bias, dtype=np.float32)

    in_maps = _in_maps(x, base_weight, jacobi_coeffs, bias)
    nc = _get_nc()
    res = run_bass_kernel_spmd(nc, in_maps, core_ids=list(range(N_CORES)))

    out = np.empty((BATCH, OUT_F), dtype=np.float32)
    for c in range(N_CORES):
        g, h = c // OUT_HALVES, c % OUT_HALVES
        out[g * B_SHARD:(g + 1) * B_SHARD,
            h * O_SHARD:(h + 1) * O_SHARD] = res.results[c]["out"]
    return out


# revision 36
# speedup vs baseline: 1.0056x; 1.0056x over previous
"""JacobiKANLinear TRN2 Bass kernel.

out = silu(x) @ W_base^T + einsum('bik,oik->bo', P(tanh(x)), C) + bias

Host-side algebra: Jacobi polynomials (A=B=1, degree 5) are re-expressed in
the monomial basis.  D[o,i,j] = sum_k C[o,i,k] * T[k,j] where T holds the
monomial coefficients of P_k.  The j=0 term is constant (t^0 == 1) and folds
into the bias.  The device then computes 6 feature blocks
[silu(x), t, t^2, t^3, t^4, t^5] (t = tanh(x)) and one fused matmul with
contraction 6*1024 = 6144.  Bias is added during the PSUM->SBUF drain via a
host-broadcast [128, O_SHARD] tile (no PSUM-init matmul).

Sharding (8 cores): 4 batch groups x 2 out-feature halves.  Per core:
batch shard 2048 rows, out shard 512 cols.  Matmuls run in float16
(1 cycle/row on the PE at N=512; fp16 weight loads get the fast-weight-load
path, and fp16 halves the weight DMA and the DVE power-chain cost).
Measured end-to-end quantization error on the actual fixed inputs is
rel 1.4e-3 (vs the 2e-2 gate); accumulation stays fp32 in PSUM.

Scheduling: the v1 kernel spent its first 45us with the PE idle — the
12.6MB weight DMA plus bias/ones DMAs (emitted last) gated every matmul.
This version overlaps the weight-DMA phase with compute: the first 4 batch
chunks' x DMAs are issued up front on the GpSimd DGE queue (x is laid out
host-side so each chunk is a contiguous 4KB-per-partition transfer), the
fp16 weights stream on the Sync queue in pairs, and the 16 chunks are
emitted chunk-major — the Tile scheduler staggers matmuls across chunks as
weight tiles and feature blocks become ready.  A short burst of dummy
matmuls right after the preamble keeps the PE busy so the HAM clock gate
un-throttles (1.2 -> 2.4 GHz) before the real matmul stream begins.
"""
import numpy as np

import concourse.bass as bass
import concourse.mybir as mybir
import concourse.tile as tile
from concourse import bacc
from concourse.bass_utils import run_bass_kernel_spmd

BATCH = 8192
IN_F = 1024
OUT_F = 1024
DEGREE = 5
A = 1.0
B = 1.0

N_CORES = 8
BATCH_GROUPS = 4
OUT_HALVES = 2
B_SHARD = BATCH // BATCH_GROUPS        # 2048
O_SHARD = OUT_F // OUT_HALVES          # 512
N_BLOCKS = DEGREE + 1                  # 6 feature blocks
N_KT = N_BLOCKS * IN_F // 128          # 48 contraction tiles of 128
N_CHUNKS = B_SHARD // 128              # 16 batch chunks per core
IT_PER_BLOCK = IN_F // 128             # 8 in-feature tiles per block
N_WARM = 4                             # chunks interleaved with the w DMA

F32 = mybir.dt.float32
F32R = mybir.dt.float32r
F16 = mybir.dt.float16
W_DMA_GROUP = 4                        # w contraction tiles per DMA


def _jacobi_monomial_matrix():
    """T[k, j] = coefficient of t^j in P_k (A=B=1), float64."""
    T = np.zeros((DEGREE + 1, DEGREE + 1), dtype=np.float64)
    polys = [np.zeros(DEGREE + 1) for _ in range(DEGREE + 1)]
    polys[0][0] = 1.0
    if DEGREE >= 1:
        # 0.5 * (2(A+1) t + (A-B))
        polys[1][1] = A + 1.0
        polys[1][0] = 0.5 * (A - B)
    for k in range(2, DEGREE + 1):
        alpha_n = 2.0 * k * (k + A + B) * (2 * k + A + B - 2)
        beta_n = (2 * k + A + B - 1) * (A ** 2 - B ** 2)
        gamma_n = (2 * k + A + B - 2) * (2 * k + A + B - 1) * (2 * k + A + B)
        delta_n = 2.0 * (k + A - 1) * (k + B - 1) * (2 * k + A + B)
        # P_k = ((beta + alpha t)/gamma) P_{k-1} - (delta/gamma) P_{k-2}
        p = np.zeros(DEGREE + 1)
        p += (beta_n / gamma_n) * polys[k - 1]
        p[1:] += (alpha_n / gamma_n) * polys[k - 1][:-1]
        p -= (delta_n / gamma_n) * polys[k - 2]
        polys[k] = p
    for k in range(DEGREE + 1):
        T[k] = polys[k]
    return T


def _build_nc():
    nc = bacc.Bacc()
    xt_in = nc.declare_dram_parameter(
        "xt", [N_CHUNKS, 128, IT_PER_BLOCK, 128], F16, isOutput=False)
    w_in = nc.declare_dram_parameter(
        "w", [128, N_KT, O_SHARD], F16, isOutput=False)
    biasb_in = nc.declare_dram_parameter(
        "biasb", [128, O_SHARD], F32, isOutput=False)
    out = nc.declare_dram_parameter("out", [B_SHARD, O_SHARD], F16, isOutput=True)

    with tile.TileContext(nc) as tc:
        with tc.tile_pool(name="wpool", bufs=1) as wpool, \
             tc.tile_pool(name="xpool", bufs=4) as xpool, \
             tc.tile_pool(name="fpool", bufs=N_WARM) as fpool, \
             tc.tile_pool(name="opool", bufs=2) as opool, \
             tc.tile_pool(name="psum", bufs=8, space="PSUM") as psum_pool:

            def x_dma(m, eng=None):
                # x/out/bias DMAs go on the (otherwise idle) GpSimd queue so
                # they never serialize behind the weight stream on Sync.
                # Exception: x0 goes FIRST on sync, where it gets the HBM to
                # itself for ~1.3us and lands before the weight stream ramps —
                # chunk 0's silu gates the start of the real matmul stream.
                x_m = xpool.tile([128, IT_PER_BLOCK, 128], F16, tag="x")
                (eng or nc.gpsimd).dma_start(out=x_m[:], in_=xt_in[m])
                return x_m

            def blocks_for(x_m):
                silu_m = fpool.tile([128, IT_PER_BLOCK, 128], F16, tag="silu")
                t_m = fpool.tile([128, IT_PER_BLOCK, 128], F16, tag="t1")
                nc.scalar.activation(
                    silu_m[:], x_m[:], mybir.ActivationFunctionType.Silu)
                nc.scalar.activation(
                    t_m[:], x_m[:], mybir.ActivationFunctionType.Tanh)
                t2_m = fpool.tile([128, IT_PER_BLOCK, 128], F16, tag="t2")
                nc.vector.tensor_mul(t2_m[:], t_m[:], t_m[:])
                t3_m = fpool.tile([128, IT_PER_BLOCK, 128], F16, tag="t3")
                nc.vector.tensor_mul(t3_m[:], t2_m[:], t_m[:])
                t4_m = fpool.tile([128, IT_PER_BLOCK, 128], F16, tag="t4")
                nc.vector.tensor_mul(t4_m[:], t3_m[:], t_m[:])
                t5_m = fpool.tile([128, IT_PER_BLOCK, 128], F16, tag="t5")
                nc.vector.tensor_mul(t5_m[:], t4_m[:], t_m[:])
                return [silu_m, t_m, t2_m, t3_m, t4_m, t5_m]

            def drain(m, ps):
                o_m = opool.tile([128, O_SHARD], F16, tag="o")
                if m == N_CHUNKS - 1:
                    # Last chunk is the kernel tail: slice the bias-add and
                    # ship halves on the (idle, HWDGE) sync queue so the out
                    # DMA overlaps the second half's add.
                    for h in range(2):
                        sl = bass.ts(h, O_SHARD // 2)
                        nc.vector.tensor_add(o_m[:, sl], ps[:, sl], biasb_sb[:, sl])
                        nc.sync.dma_start(
                            out=out[bass.ts(m, 128), sl], in_=o_m[:, sl])
                else:
                    nc.vector.tensor_add(o_m[:], ps[:], biasb_sb[:])
                    nc.gpsimd.dma_start(out=out[bass.ts(m, 128), :], in_=o_m[:])

            # --- PE warm-up operands: memset first so the dummy matmuls can
            # start the moment the PE's preamble ends.
            dum_l = wpool.tile([128, 128], F16)
            dum_r = wpool.tile([128, O_SHARD], F16)
            nc.vector.memset(dum_l[:], 0.0)
            nc.vector.memset(dum_r[:], 0.0)

            # --- DMA issue order.  Early HBM bandwidth is round-robined
            # across everything in flight, so the startup DMAs are all queued
            # on sync in strict NEED order: x0 (gates silu0 -> first matmul),
            # w0/w1, then w pairs with x1..x3 interleaved where the ACT
            # pipeline will want them (silu_c starts ~2.3us after silu_{c-1}).
            # biasb — not needed until the first drain (~25us) — goes last.
            # Single sync ring in strict need-order: chunk 0 alone consumes a
            # 4-tile w group every ~1.7us, which is the full ring rate — so
            # nothing else may transfer until the first three w groups are
            # in.  x1..x3 only gate chunks 1-3 joining (the ACT stagger no
            # longer binds once chunk 0 is streaming).
            w_sb = wpool.tile([128, N_KT, O_SHARD], F16)
            x_tiles = {}
            x_tiles[0] = x_dma(0, eng=nc.sync)
            nc.sync.dma_start(out=w_sb[:, 0:4, :], in_=w_in[:, 0:4, :])
            x_tiles[1] = x_dma(1, eng=nc.sync)
            nc.sync.dma_start(out=w_sb[:, 4:8, :], in_=w_in[:, 4:8, :])
            nc.sync.dma_start(out=w_sb[:, 8:12, :], in_=w_in[:, 8:12, :])
            x_tiles[2] = x_dma(2, eng=nc.sync)
            nc.sync.dma_start(out=w_sb[:, 12:16, :], in_=w_in[:, 12:16, :])
            nc.sync.dma_start(out=w_sb[:, 16:20, :], in_=w_in[:, 16:20, :])
            x_tiles[3] = x_dma(3, eng=nc.sync)
            for kt0 in range(20, N_KT, W_DMA_GROUP):
                nc.sync.dma_start(
                    out=w_sb[:, kt0:kt0 + W_DMA_GROUP, :],
                    in_=w_in[:, kt0:kt0 + W_DMA_GROUP, :])
            biasb_sb = wpool.tile([128, O_SHARD], F32)
            nc.sync.dma_start(out=biasb_sb[:], in_=biasb_in[:])

            # --- PE warm-up: dummy matmuls (K=128, N=512 — full-array
            # activity, which is what the HAM monitors; K=1 dummies don't
            # register) bridge from the end of the preamble to the first real
            # matmul so the HAM clock gate un-throttles (1.2 -> 2.4 GHz)
            # before the real stream begins.
            ps_warm = psum_pool.tile([128, O_SHARD], F32, tag="ps")
            for _ in range(12):
                nc.tensor.matmul(
                    ps_warm[:], dum_l[:], dum_r[:], start=True, stop=True)

            # --- Phase A: warm chunks, staggered-diagonal order.  Chunk c's
            # activations finish ~2.3us after chunk c-1's (ACT is serial), and
            # weight tiles land at ~0.4us each, so emitting matmuls sorted by
            # kt + 5*c keeps the PE stream's static order feasible: chunk 0
            # runs ~5 tiles ahead of chunk 1, etc.
            warm_blocks = []
            warm_ps = []
            for c in range(N_WARM):
                warm_blocks.append(blocks_for(x_tiles[c]))
                ps_c = psum_pool.tile([128, O_SHARD], F32, tag="ps")
                warm_ps.append(ps_c)
            order = sorted(
                ((kt + 5 * c, c, kt) for c in range(N_WARM) for kt in range(N_KT)))
            for _, c, kt in order:
                b, it = kt // IT_PER_BLOCK, kt % IT_PER_BLOCK
                nc.tensor.matmul(
                    warm_ps[c][:], warm_blocks[c][b][:, it, :], w_sb[:, kt, :],
                    start=(kt == 0), stop=(kt == N_KT - 1))
            for c in range(N_WARM):
                drain(c, warm_ps[c])

            # --- Phase B: remaining chunks, chunk-major (weights resident).
            for m in range(N_WARM, N_CHUNKS):
                x_m = x_dma(m)
                blocks = blocks_for(x_m)
                ps = psum_pool.tile([128, O_SHARD], F32, tag="ps")
                for kt in range(N_KT):
                    b, it = kt // IT_PER_BLOCK, kt % IT_PER_BLOCK
                    nc.tensor.matmul(
                        ps[:], blocks[b][:, it, :], w_sb[:, kt, :],
                        start=(kt == 0), stop=(kt == N_KT - 1))
                drain(m, ps)
    nc.finalize()
    return nc


_NC_CACHE = None


def _get_nc():
    global _NC_CACHE
    if _NC_CACHE is None:
        _NC_CACHE = _build_nc()
    return _NC_CACHE


def _prepare_host(x, base_weight, jacobi_coeffs, bias):
    T = _jacobi_monomial_matrix()
    D = np.einsum("oik,kj->oij", jacobi_coeffs.astype(np.float64), T)
    bias_eff = bias.astype(np.float64) + D[:, :, 0].sum(axis=1)

    # W'[f, o]: 6 blocks of IN_F feature rows: silu -> base_weight, t^j -> D_j
    w_full = np.empty((N_BLOCKS * IN_F, OUT_F), dtype=np.float32)
    w_full[0:IN_F] = base_weight.T
    for j in range(1, N_BLOCKS):
        w_full[j * IN_F:(j + 1) * IN_F] = D[:, :, j].T.astype(np.float32)

    w_halves = []
    biasb_halves = []
    for h in range(OUT_HALVES):
        wh = w_full[:, h * O_SHARD:(h + 1) * O_SHARD]
        # SBUF layout [128, N_KT, O_SHARD]: [p, kt, n] = wh[kt*128 + p, n]
        wh = np.ascontiguousarray(
            wh.reshape(N_KT, 128, O_SHARD).transpose(1, 0, 2).astype(np.float16))
        w_halves.append(wh)
        bh = bias_eff[h * O_SHARD:(h + 1) * O_SHARD].astype(np.float32)
        biasb_halves.append(np.ascontiguousarray(
            np.broadcast_to(bh[None, :], (128, O_SHARD))))

    xt_groups = []
    for g in range(BATCH_GROUPS):
        xs = x[g * B_SHARD:(g + 1) * B_SHARD]              # (B_SHARD, IN_F)
        # [m, p, it, b] = xs[m*128 + b, it*128 + p]: per (m, p) the 4KB
        # [it, b] plane is contiguous, so each chunk DMA is 128 fat
        # descriptors instead of 1024 strided 512B ones.
        xt = np.ascontiguousarray(
            xs.reshape(N_CHUNKS, 128, IT_PER_BLOCK, 128)
            .transpose(0, 3, 2, 1).astype(np.float16))
        xt_groups.append(xt)
    return xt_groups, w_halves, biasb_halves


def _in_maps(x, base_weight, jacobi_coeffs, bias):
    xt_groups, w_halves, biasb_halves = _prepare_host(
        x, base_weight, jacobi_coeffs, bias)
    maps = []
    for c in range(N_CORES):
        g, h = c // OUT_HALVES, c % OUT_HALVES
        maps.append({
            "xt": xt_groups[g],
            "w": w_halves[h],
            "biasb": biasb_halves[h],
        })
    return maps


def kernel(x, base_weight, jacobi_coeffs, bias):
    x = np.asarray(x, dtype=np.float32)
    base_weight = np.asarray(base_weight, dtype=np.float32)
    jacobi_coeffs = np.asarray(jacobi_coeffs, dtype=np.float32)
    bias = np.asarray(bias, dtype=np.float32)

    in_maps = _in_maps(x, base_weight, jacobi_coeffs, bias)
    nc = _get_nc()
    res = run_bass_kernel_spmd(nc, in_maps, core_ids=list(range(N_CORES)))

    out = np.empty((BATCH, OUT_F), dtype=np.float32)
    for c in range(N_CORES):
        g, h = c // OUT_HALVES, c % OUT_HALVES
        out[g * B_SHARD:(g + 1) * B_SHARD,
            h * O_SHARD:(h + 1) * O_SHARD] = res.results[c]["out"]
    return out
